# revision 4
# baseline (speedup 1.0000x reference)
"""Trainium2 Bass kernel for KBRD-style RGCN message passing + attention
pooling + full-entity scoring, SPMD over 8 NeuronCores.

Pipeline (per core, nodes dst-sharded into contiguous ranges):
  Phase A: for each dst tile (128 nodes), accumulate in PSUM over edge
    blocks of 128: gather basis rows of the 128 src nodes (indirect DMA,
    one [NB*DIM]-wide row per edge), build "weighted one-hot" matrices
    W_b[e, j] = (dstloc[e]==j) * att[type_e, b] on DVE, and PE-matmul
    W_b^T @ X_b accumulating agg[dst, dim].  Add root (+rgcn_bias via a
    K=1 ones-matmul).  Store nodes shard to DRAM and its transpose to
    SBUF for scoring.
  Phase B: gather the seed rows this core owns from its nodes shard
    (indirect DMA with OOB-skip for rows owned by other cores, zeros
    elsewhere), AllReduce [B*L, DIM] to assemble full h everywhere.
    Attention pooling computed redundantly per core: e = tanh(h@A)@b,
    masked softmax over L, u = attn-weighted sum of h.
  Phase C: scores_shard = u @ nodes_shard^T + bias (PE), plus row max
    and sum(exp(x-max)) partials for the loss logsumexp.
Host combines shards: concat scores, logsumexp-combine partials, loss.
"""
import numpy as np
from contextlib import ExitStack

import concourse.bass as bass
import concourse.tile as tile
from concourse import mybir
from concourse.bass_utils import run_bass_kernel_spmd
from concourse.vector_clock import ScopedClock

# ---------------------------------------------------------------- patch
# Walrus in this container accepts at most ONE sem wait per instruction.
# Tile's scheduler can attach several; split extras onto dedicated NoOps.
_PATCHED = False


def _apply_walrus_patch():
    global _PATCHED
    if _PATCHED:
        return
    _PATCHED = True
    _orig_add = tile.TileContext._add_instruction

    def _add_instruction(self, inst):
        si = getattr(inst, "sync_info", None)
        eng = getattr(inst, "engine", None)
        if (
            si is not None
            and si.on_wait
            and len(si.on_wait) > 1
            and eng is not None
            and eng != mybir.EngineType.Unassigned
        ):
            waits = list(si.on_wait)
            for w in waits[:-1]:
                nop = mybir.InstNoOp(
                    name=f"I-{self.nc.next_id()}",
                    sync_info=mybir.SyncInfo(on_wait=[w], on_update=[]),
                    bass_nofuse=True,
                    engine=eng,
                )
                _orig_add(self, nop)
            si.on_wait = waits[-1:]
        _orig_add(self, inst)

    def _drain_and_barrier(self, tick_clock, wait_clock):
        nop0 = self.nc.sync.nop(nofuse=True, hint="tail_wait_nop")
        wait_clock.add_sem_waits(
            nop0.ins, ScopedClock({None: tick_clock.global_clock})
        )
        waits = list(nop0.ins.sync_info.on_wait or [])
        if len(waits) > 1:
            nop0.ins.sync_info.on_wait = waits[:1]
            for i in range(1, len(waits)):
                n = self.nc.sync.nop(nofuse=True, hint="tail_wait_nop")
                n.ins.sync_info = mybir.SyncInfo(
                    on_wait=waits[i:i + 1], on_update=[]
                )
        self.nc.all_engine_barrier()
        assert self.sems is not None
        popped = self.nc._tile_sem_poison_stack.pop()
        assert popped is self._sem_poison
        self.nc.clear_and_free_semaphores(list(self.sems.allocated().values()))
        self.nc.all_engine_barrier()

    tile.TileContext._add_instruction = _add_instruction
    tile.TileContext._drain_and_barrier = _drain_and_barrier


# ---------------------------------------------------------------- config
F32 = mybir.dt.float32
I32 = mybir.dt.int32
NEG_BIG = -1.0e30


class Cfg:
    def __init__(self, n_entity, n_rel, dim, nb, batch, maxseed, n_cores,
                 use_bf16=True):
        assert dim == 128
        self.N = n_entity
        self.NREL = n_rel
        self.DIM = dim
        self.NB = nb
        self.B = batch
        self.L = maxseed
        self.NC = n_cores
        # pad N so each core's shard is a multiple of 128
        self.NPAD = ((n_entity + n_cores * 128 - 1) // (n_cores * 128)) * (n_cores * 128)
        self.SHARD = self.NPAD // n_cores
        self.NT = self.SHARD // 128
        self.NSEED = batch * maxseed
        assert self.NSEED % 128 == 0
        self.NSCH = self.NSEED // 128          # seed chunks of 128
        self.PERCH = 128 // maxseed            # samples per seed chunk
        assert self.PERCH * maxseed == 128
        assert batch % self.PERCH == 0
        # score chunk: largest divisor of SHARD that is <= 512
        ch = 512
        while self.SHARD % ch:
            ch -= 1
        self.CH = ch
        self.NCH = self.SHARD // ch
        self.BDT = mybir.dt.bfloat16 if use_bf16 else F32
        self.ROWW = nb * dim                   # basisAll row width


FULL = dict(n_entity=64368, n_rel=40, dim=128, nb=8, batch=64, maxseed=32,
            n_cores=8)


# ---------------------------------------------------------------- device
def build_program(cfg: Cfg, K: int):
    """K = uniform number of 128-edge blocks per dst tile (data-dependent)."""
    _apply_walrus_patch()
    nc = bass.Bass("TRN2", target_bir_lowering=False, debug=False,
                   num_devices=cfg.NC)
    DIM, NB, B, L = cfg.DIM, cfg.NB, cfg.B, cfg.L
    NT, SHARD, NSCH, PERCH = cfg.NT, cfg.SHARD, cfg.NSCH, cfg.PERCH
    CH, NCH, BDT, ROWW = cfg.CH, cfg.NCH, cfg.BDT, cfg.ROWW

    dt_in = lambda name, shape, dt: nc.dram_tensor(name, shape, dt, kind="ExternalInput").ap()
    dt_out = lambda name, shape, dt: nc.dram_tensor(name, shape, dt, kind="ExternalOutput").ap()

    basisAll = dt_in("basisAll", [cfg.NPAD, ROWW], BDT)
    esrc_d = dt_in("esrc", [NT * 128, K], I32)
    edloc_d = dt_in("edloc", [NT * 128, K], F32)
    ecoef_d = dt_in("ecoef", [NT * 128, K * NB], F32)
    root_d = dt_in("root", [SHARD, DIM], F32)
    bias_d = dt_in("obias", [1, SHARD], F32)
    seedsel_d = dt_in("seedsel", [cfg.NSEED, 1], I32)
    maskadd_d = dt_in("maskadd", [B, L], F32)
    attnA_d = dt_in("attnA", [DIM, DIM], F32)
    attnb_d = dt_in("attnb", [DIM, 1], F32)
    rgcnb_d = dt_in("rgcnb", [1, DIM], F32)
    iota_d = dt_in("iotaf", [128, 128], F32)
    ident_d = dt_in("ident", [128, 128], F32)
    oh4_d = dt_in("oh4", [128, PERCH], F32)

    scores_d = dt_out("scores", [B, SHARD], F32)
    rowmax_d = dt_out("rowmax", [B, 1], F32)
    rowsum_d = dt_out("rowsum", [B, 1], F32)

    nodes_dram = nc.dram_tensor("nodes_i", [SHARD, DIM], F32).ap()
    hin_dram = nc.dram_tensor("hin_i", [cfg.NSEED, DIM], F32).ap()
    hout_dram = nc.dram_tensor("hout_i", [cfg.NSEED, DIM], F32,
                               addr_space="Shared").ap()
    ert_dram = nc.dram_tensor("ert_i", [cfg.NSEED, 1], F32).ap()
    art_dram = nc.dram_tensor("art_i", [cfg.NSEED, 1], F32).ap()

    with tile.TileContext(nc) as tc, ExitStack() as ctx:
        const = ctx.enter_context(tc.tile_pool(name="const", bufs=1))
        meta = ctx.enter_context(tc.tile_pool(name="meta", bufs=2))
        xp = ctx.enter_context(tc.tile_pool(name="xp", bufs=3))
        wp = ctx.enter_context(tc.tile_pool(name="wp", bufs=3))
        ohp = ctx.enter_context(tc.tile_pool(name="ohp", bufs=3))
        np_ = ctx.enter_context(tc.tile_pool(name="np", bufs=2))
        big = ctx.enter_context(tc.tile_pool(name="big", bufs=1))
        ps = ctx.enter_context(tc.tile_pool(name="ps", bufs=4, space="PSUM"))

        # ---- constants
        iota_sb = const.tile([128, 128], F32)
        nc.sync.dma_start(out=iota_sb[:], in_=iota_d[:])
        ident_sb = const.tile([128, 128], F32)
        nc.sync.dma_start(out=ident_sb[:], in_=ident_d[:])
        rgcnb_sb = const.tile([1, DIM], F32)
        nc.sync.dma_start(out=rgcnb_sb[:], in_=rgcnb_d[:])
        onesP = const.tile([1, 128], F32)
        nc.vector.memset(onesP[:], 1.0)
        onesB = const.tile([1, B], F32)
        nc.vector.memset(onesB[:], 1.0)
        oh4_sb = const.tile([128, PERCH], F32)
        nc.sync.dma_start(out=oh4_sb[:], in_=oh4_d[:])

        # persistent transposed nodes for scoring
        nodesT = big.tile([128, SHARD], F32)

        # ---------------- Phase A: message passing ----------------
        for t in range(NT):
            esrc_sb = meta.tile([128, K], I32, tag="esrc")
            nc.sync.dma_start(out=esrc_sb[:], in_=esrc_d[t * 128:(t + 1) * 128, :])
            edloc_sb = meta.tile([128, K], F32, tag="edloc")
            nc.sync.dma_start(out=edloc_sb[:], in_=edloc_d[t * 128:(t + 1) * 128, :])
            ecoef_sb = meta.tile([128, K * NB], F32, tag="ecoef")
            nc.sync.dma_start(out=ecoef_sb[:], in_=ecoef_d[t * 128:(t + 1) * 128, :])
            root_sb = meta.tile([128, DIM], F32, tag="root")
            nc.sync.dma_start(out=root_sb[:], in_=root_d[t * 128:(t + 1) * 128, :])

            agg = ps.tile([128, DIM], F32, tag="ps")
            # rgcn_bias broadcast to every dst row (K=1 matmul), starts group
            nc.tensor.matmul(out=agg[:], lhsT=onesP[:], rhs=rgcnb_sb[:],
                             start=True, stop=False)
            for k in range(K):
                x = xp.tile([128, ROWW], BDT, tag="x")
                nc.gpsimd.indirect_dma_start(
                    out=x[:], out_offset=None,
                    in_=basisAll[:],
                    in_offset=bass.IndirectOffsetOnAxis(
                        ap=esrc_sb[:, k:k + 1], axis=0),
                )
                w = wp.tile([128, ROWW], BDT, tag="w")
                for b in range(NB):
                    nc.vector.tensor_scalar(
                        out=w[:, b * DIM:(b + 1) * DIM],
                        in0=iota_sb[:],
                        scalar1=edloc_sb[:, k:k + 1],
                        scalar2=ecoef_sb[:, k * NB + b:k * NB + b + 1],
                        op0=mybir.AluOpType.is_equal,
                        op1=mybir.AluOpType.mult,
                    )
                for b in range(NB):
                    nc.tensor.matmul(
                        out=agg[:],
                        lhsT=w[:, b * DIM:(b + 1) * DIM],
                        rhs=x[:, b * DIM:(b + 1) * DIM],
                        start=False,
                        stop=(k == K - 1 and b == NB - 1),
                    )
            nodes_sb = np_.tile([128, DIM], F32, tag="nodes")
            nc.vector.tensor_add(out=nodes_sb[:], in0=agg[:], in1=root_sb[:])
            nc.sync.dma_start(out=nodes_dram[t * 128:(t + 1) * 128, :],
                              in_=nodes_sb[:])
            ptr = ps.tile([128, 128], F32, tag="ps")
            nc.tensor.transpose(out=ptr[:], in_=nodes_sb[:], identity=ident_sb[:])
            nc.scalar.copy(out=nodesT[:, t * 128:(t + 1) * 128], in_=ptr[:])

        # ---------------- Phase B: seed gather + attention pooling
        for c in range(NSCH):
            sel_sb = meta.tile([128, 1], I32, tag="sel")
            nc.sync.dma_start(out=sel_sb[:],
                              in_=seedsel_d[c * 128:(c + 1) * 128, :])
            hp = np_.tile([128, DIM], F32, tag="hp")
            nc.vector.memset(hp[:], 0.0)
            nc.gpsimd.indirect_dma_start(
                out=hp[:], out_offset=None,
                in_=nodes_dram[:],
                in_offset=bass.IndirectOffsetOnAxis(ap=sel_sb[:, :1], axis=0),
                bounds_check=SHARD - 1,
                oob_is_err=False,
            )
            nc.sync.dma_start(out=hin_dram[c * 128:(c + 1) * 128, :], in_=hp[:])

        nc.gpsimd.collective_compute(
            "AllReduce", mybir.AluOpType.add,
            ins=[hin_dram[:]], outs=[hout_dram[:]],
            replica_groups=[list(range(cfg.NC))],
        )

        h_sb = big.tile([128, NSCH * DIM], F32)      # seed rows, chunk-major
        hT_sb = big.tile([128, NSCH * 128], F32)     # transposed: [dim, seed]
        for c in range(NSCH):
            nc.sync.dma_start(out=h_sb[:, c * DIM:(c + 1) * DIM],
                              in_=hout_dram[c * 128:(c + 1) * 128, :])
            pt = ps.tile([128, 128], F32, tag="ps")
            nc.tensor.transpose(out=pt[:], in_=h_sb[:, c * DIM:(c + 1) * DIM],
                                identity=ident_sb[:])
            nc.scalar.copy(out=hT_sb[:, c * 128:(c + 1) * 128], in_=pt[:])

        attnA_sb = const.tile([DIM, DIM], F32)
        nc.sync.dma_start(out=attnA_sb[:], in_=attnA_d[:])
        attnb_sb = const.tile([DIM, 1], F32)
        nc.sync.dma_start(out=attnb_sb[:], in_=attnb_d[:])

        tanhT = big.tile([128, cfg.NSEED], F32)
        nq = (cfg.NSEED + 511) // 512
        for q in range(nq):
            n0, n1 = q * 512, min((q + 1) * 512, cfg.NSEED)
            ptq = ps.tile([128, 512], F32, tag="ps")
            nc.tensor.matmul(out=ptq[:, :n1 - n0], lhsT=attnA_sb[:],
                             rhs=hT_sb[:, n0:n1], start=True, stop=True)
            nc.scalar.activation(out=tanhT[:, n0:n1], in_=ptq[:, :n1 - n0],
                                 func=mybir.ActivationFunctionType.Tanh)
        e_sb = np_.tile([1, cfg.NSEED], F32, tag="e")
        for q in range(nq):
            n0, n1 = q * 512, min((q + 1) * 512, cfg.NSEED)
            pe = ps.tile([1, 512], F32, tag="ps")
            nc.tensor.matmul(out=pe[:, :n1 - n0], lhsT=attnb_sb[:],
                             rhs=tanhT[:, n0:n1], start=True, stop=True)
            nc.vector.tensor_copy(out=e_sb[:, n0:n1], in_=pe[:, :n1 - n0])
        nc.sync.dma_start(
            out=ert_dram.rearrange("(a n) o -> a (n o)", a=1)[:],
            in_=e_sb[:])

        e64 = np_.tile([B, L], F32, tag="e64")
        nc.sync.dma_start(
            out=e64[:],
            in_=ert_dram.rearrange("(b l) o -> b (l o)", b=B)[:])
        madd_sb = meta.tile([B, L], F32, tag="madd")
        nc.sync.dma_start(out=madd_sb[:], in_=maskadd_d[:])
        nc.vector.tensor_add(out=e64[:], in0=e64[:], in1=madd_sb[:])
        em = np_.tile([B, 1], F32, tag="em")
        nc.vector.reduce_max(out=em[:], in_=e64[:], axis=mybir.AxisListType.X)
        emn = np_.tile([B, 1], F32, tag="emn")
        nc.vector.tensor_scalar_mul(emn[:], em[:], -1.0)
        p64 = np_.tile([B, L], F32, tag="p64")
        nc.scalar.activation(out=p64[:], in_=e64[:],
                             func=mybir.ActivationFunctionType.Exp,
                             bias=emn[:, :1])
        es = np_.tile([B, 1], F32, tag="es")
        nc.vector.reduce_sum(out=es[:], in_=p64[:], axis=mybir.AxisListType.X)
        esr = np_.tile([B, 1], F32, tag="esr")
        nc.vector.reciprocal(esr[:], es[:])
        attn64 = np_.tile([B, L], F32, tag="attn64")
        nc.vector.tensor_scalar_mul(attn64[:], p64[:], esr[:, :1])
        nc.sync.dma_start(
            out=art_dram.rearrange("(b l) o -> b (l o)", b=B)[:],
            in_=attn64[:])

        # u = sum_l attn * h   (per seed chunk: PERCH samples; weights are
        # placed in columns c*PERCH.. of a zeroed [128, B] lhsT so every
        # chunk's matmul accumulates into one [B, DIM] psum at base 0)
        pu = ps.tile([B, DIM], F32, tag="ps")
        for c in range(NSCH):
            af = meta.tile([128, 1], F32, tag="af")
            nc.sync.dma_start(out=af[:], in_=art_dram[c * 128:(c + 1) * 128, :])
            wuf = meta.tile([128, B], F32, tag="wu")
            nc.vector.memset(wuf[:], 0.0)
            nc.vector.tensor_scalar_mul(
                wuf[:, c * PERCH:(c + 1) * PERCH], oh4_sb[:], af[:, :1])
            nc.tensor.matmul(out=pu[:], lhsT=wuf[:],
                             rhs=h_sb[:, c * DIM:(c + 1) * DIM],
                             start=(c == 0), stop=(c == NSCH - 1))
        u_sb = np_.tile([B, DIM], F32, tag="u")
        nc.vector.tensor_copy(out=u_sb[:], in_=pu[:])
        put = ps.tile([DIM, B], F32, tag="ps")
        nc.tensor.transpose(out=put[:], in_=u_sb[:],
                            identity=ident_sb[:B, :B])
        uT_sb = np_.tile([DIM, B], F32, tag="uT")
        nc.vector.tensor_copy(out=uT_sb[:], in_=put[:])

        # ---------------- Phase C: scoring + softmax partials
        bias_sb = big.tile([1, SHARD], F32)
        nc.sync.dma_start(out=bias_sb[:], in_=bias_d[:])
        scores_sb = big.tile([B, SHARD], F32)
        for q in range(NCH):
            n0, n1 = q * CH, (q + 1) * CH
            psc = ps.tile([B, CH], F32, tag="ps")
            nc.tensor.matmul(out=psc[:], lhsT=uT_sb[:], rhs=nodesT[:, n0:n1],
                             start=True, stop=False)
            nc.tensor.matmul(out=psc[:], lhsT=onesB[:], rhs=bias_sb[:, n0:n1],
                             start=False, stop=True)
            nc.scalar.copy(out=scores_sb[:, n0:n1], in_=psc[:])
        nc.sync.dma_start(out=scores_d[:], in_=scores_sb[:])
        mx = np_.tile([B, 1], F32, tag="mx")
        nc.vector.reduce_max(out=mx[:], in_=scores_sb[:],
                             axis=mybir.AxisListType.X)
        mxn = np_.tile([B, 1], F32, tag="mxn")
        nc.vector.tensor_scalar_mul(mxn[:], mx[:], -1.0)
        expv = big.tile([B, SHARD], F32)
        nc.scalar.activation(out=expv[:], in_=scores_sb[:],
                             func=mybir.ActivationFunctionType.Exp,
                             bias=mxn[:, :1])
        sm = np_.tile([B, 1], F32, tag="sm")
        nc.vector.reduce_sum(out=sm[:], in_=expv[:], axis=mybir.AxisListType.X)
        nc.sync.dma_start(out=rowmax_d[:], in_=mx[:])
        nc.sync.dma_start(out=rowsum_d[:], in_=sm[:])

    return nc


# ---------------------------------------------------------------- host
def prep_inputs(cfg: Cfg, basis, att, root, rgcn_bias, attn_a, attn_b,
                output_bias, edge_index, edge_type, seed_idx, seed_mask):
    """Shard + lay out inputs for the SPMD program. Returns (in_maps, K)."""
    N, NB, DIM, NC, SHARD, NT = cfg.N, cfg.NB, cfg.DIM, cfg.NC, cfg.SHARD, cfg.NT
    B, L = cfg.B, cfg.L

    basis = np.asarray(basis, dtype=np.float32)
    att = np.asarray(att, dtype=np.float32)
    root = np.asarray(root, dtype=np.float32)
    rgcn_bias = np.asarray(rgcn_bias, dtype=np.float32)
    attn_a = np.asarray(attn_a, dtype=np.float32)
    attn_b = np.asarray(attn_b, dtype=np.float32)
    output_bias = np.asarray(output_bias, dtype=np.float32)
    edge_index = np.asarray(edge_index)
    edge_type = np.asarray(edge_type)
    seed_idx = np.asarray(seed_idx).astype(np.int64)
    seed_mask = np.asarray(seed_mask).astype(bool)

    # basisAll [NPAD, NB*DIM]
    basisAll = np.zeros((cfg.NPAD, NB * DIM), dtype=np.float32)
    basisAll[:N] = basis.transpose(1, 0, 2).reshape(N, NB * DIM)
    if cfg.BDT == mybir.dt.bfloat16:
        import ml_dtypes
        basisAll = basisAll.astype(ml_dtypes.bfloat16)

    src = edge_index[0].astype(np.int64)
    dst = edge_index[1].astype(np.int64)
    et = edge_type.astype(np.int64)
    coeff = att[et]                                   # [E, NB]

    core = dst // SHARD
    tl = (dst % SHARD) // 128
    loc = dst % 128

    # per-(core,tile) edge counts -> uniform K
    counts = np.zeros((NC, NT), dtype=np.int64)
    np.add.at(counts, (core, tl), 1)
    K = max(1, int(np.ceil(counts.max() / 128)))

    in_maps = []
    order = np.lexsort((tl, core))                    # group by (core, tile)
    src_s, tl_s, loc_s, coeff_s, core_s = (
        src[order], tl[order], loc[order], coeff[order], core[order])
    core_starts = np.searchsorted(core_s, np.arange(NC + 1))

    seed_flat = seed_idx.reshape(-1)                  # [NSEED]
    maskadd = np.where(seed_mask, 0.0, NEG_BIG).astype(np.float32)

    iota = np.broadcast_to(np.arange(128, dtype=np.float32), (128, 128)).copy()
    ident = np.eye(128, dtype=np.float32)
    oh4 = np.zeros((128, cfg.PERCH), dtype=np.float32)
    for p in range(128):
        oh4[p, p // L] = 1.0

    for c in range(NC):
        s0, s1 = core_starts[c], core_starts[c + 1]
        ctl, cloc, csrc, ccoef = tl_s[s0:s1], loc_s[s0:s1], src_s[s0:s1], coeff_s[s0:s1]
        esrc = np.zeros((NT, K * 128), dtype=np.int32)
        edloc = np.zeros((NT, K * 128), dtype=np.float32)
        ecoef = np.zeros((NT, K * 128, NB), dtype=np.float32)
        t_starts = np.searchsorted(ctl, np.arange(NT + 1))
        for t in range(NT):
            a, b_ = t_starts[t], t_starts[t + 1]
            n = b_ - a
            esrc[t, :n] = csrc[a:b_]
            edloc[t, :n] = cloc[a:b_]
            ecoef[t, :n] = ccoef[a:b_]
        # [NT, K*128] -> [NT, K, 128] -> [NT, 128, K] -> [NT*128, K]
        esrc = esrc.reshape(NT, K, 128).transpose(0, 2, 1).reshape(NT * 128, K)
        edloc = edloc.reshape(NT, K, 128).transpose(0, 2, 1).reshape(NT * 128, K)
        ecoef = (ecoef.reshape(NT, K, 128, NB).transpose(0, 2, 1, 3)
                 .reshape(NT * 128, K * NB))

        rootc = np.zeros((SHARD, DIM), dtype=np.float32)
        lo, hi = c * SHARD, min((c + 1) * SHARD, N)
        if hi > lo:
            rootc[:hi - lo] = root[lo:hi]
        biasc = np.full((1, SHARD), NEG_BIG, dtype=np.float32)
        if hi > lo:
            biasc[0, :hi - lo] = output_bias[lo:hi]

        owner = seed_flat // SHARD
        # sentinel SHARD is OOB (bounds_check=SHARD-1) but small enough that
        # index*row_stride cannot overflow int32 in the DGE index arithmetic
        sel = np.where(owner == c, seed_flat % SHARD, SHARD).astype(np.int32)

        in_maps.append(dict(
            basisAll=basisAll,
            esrc=np.ascontiguousarray(esrc),
            edloc=np.ascontiguousarray(edloc),
            ecoef=np.ascontiguousarray(ecoef),
            root=rootc, obias=biasc,
            seedsel=sel[:, None],
            maskadd=maskadd,
            attnA=attn_a, attnb=attn_b,
            rgcnb=rgcn_bias[None, :],
            iotaf=iota, ident=ident, oh4=oh4,
        ))
    return in_maps, K


def combine_outputs(cfg: Cfg, results, labels):
    labels = np.asarray(labels).astype(np.int64)
    scores_full = np.concatenate([r["scores"] for r in results], axis=1)
    scores = scores_full[:, :cfg.N]
    rm = np.stack([r["rowmax"][:, 0] for r in results], axis=1).astype(np.float64)
    rs = np.stack([r["rowsum"][:, 0] for r in results], axis=1).astype(np.float64)
    gmax = rm.max(axis=1)
    tot = (rs * np.exp(rm - gmax[:, None])).sum(axis=1)
    lse = gmax + np.log(tot)
    lab = scores[np.arange(cfg.B), labels].astype(np.float64)
    loss = np.float32(-(lab - lse).mean())
    return scores, loss


_CACHE = {}


def run_pipeline(cfg: Cfg, inputs, trace=False, trace_kwargs=None):
    in_maps, K = prep_inputs(
        cfg, inputs["basis"], inputs["att"], inputs["root"],
        inputs["rgcn_bias"], inputs["attn_a"], inputs["attn_b"],
        inputs["output_bias"], inputs["edge_index"], inputs["edge_type"],
        inputs["seed_idx"], inputs["seed_mask"])
    key = (id(type(cfg)), cfg.N, cfg.NC, cfg.BDT, K)
    nc = _CACHE.get(key)
    if nc is None:
        nc = build_program(cfg, K)
        _CACHE[key] = nc
    res = run_bass_kernel_spmd(
        nc, in_maps, core_ids=list(range(cfg.NC)), trace=trace,
        **(trace_kwargs or {}))
    scores, loss = combine_outputs(cfg, res.results, inputs["labels"])
    return scores, loss, res


def kernel(**inputs):
    cfg = Cfg(**FULL)
    scores, loss, _ = run_pipeline(cfg, inputs)
    return scores, loss
